# revision 16
# baseline (speedup 1.0000x reference)
"""Trainium2 Bass kernel for a 2-layer GCN (NextHopGNN).

Distribution: destination-node sharding across 8 NeuronCores. Each core owns
N/8 destination nodes and all edges pointing into them. Per layer:
  1. each core computes its slice of the scaled feature table
     y = dinv * (x @ W) in bf16 (PE matmuls, batched 8-tile PSUM groups)
  2. AllGather (bf16) -> every core holds the full [N, 64] table in HBM
  3. per 128-node dst tile: dma_gather the edge source rows as 128-byte
     bf16 descriptors (table viewed as [N/2, 256B] token pairs; src&1
     selects a 64-column half; int16 idx = (src>>1) - 32768), build
     one-hot scatter matrices on the vector engine (batched iota ==
     dstloc), and accumulate S^T @ G in PSUM on the tensor engine.
     Self-loop added via an identity matmul from the resident bf16 table.
Layer-1 epilogue also computes the layer-2 table tile (relu -> transpose ->
@W2 -> scale) so layer 2 only needs the second AllGather + aggregation.

The SWDGE gather drain is descriptor-rate-bound (~8ns/descriptor/queue,
4 queues), so the schedule minimizes padded descriptor count (2 parity
buckets/tile) and keeps deep gather lookahead so the PE never starves.
"""
import sys
import os
import numpy as np
import ml_dtypes

sys.path.insert(0, "/opt/trn_rl_repo")

P = 128
H = 64
EDIM = 128
NCORES = 8
NPAR = 2            # src&1 parity buckets
NSTR = 4            # gather streams (= SWDGE queues), 2 per parity
ABASE = 32768       # token bias: idx = (src>>1) - ABASE, tokens N/2 = 50000
GCHUNK = 4          # chunks (of 128 idxs) per dma_gather instruction
                    # (>1024 idxs per instruction crashes the SWDGE ucode)
SBATCH = 8          # chunks per batched one-hot build
WB = 8              # tiles per batched table-build / write group

_COMPILED = {}


def _ceil_div(a, b):
    return (a + b - 1) // b


def my_dma_gather(g, out_ap, in_ap, idxs_ap, num_idxs, num_idxs_reg,
                  elem_size, elem_step, queue_num=0):
    """Replica of BassGpSimd.dma_gather (non-transpose, DRAM source) minus
    the elem_size_bytes%256 assert. The real encoding constraint is the
    256B-aligned source row stride (stride_bytes_256); 128B (bf16 x64) and
    64B (fp8 x64) elem sizes were verified bit-exact on HW."""
    from concourse import mybir, ap_utils
    from concourse.bass import exact_div, round_up_to_multiple
    assert idxs_ap.dtype == mybir.dt.int16
    assert in_ap.dtype == out_ap.dtype
    assert ap_utils.ap_is_contiguous(out_ap.ap[1:])
    assert ap_utils.ap_is_contiguous(idxs_ap.ap[1:])
    assert in_ap.ap[-1][1] == out_ap.ap[-1][1] == elem_size
    assert out_ap.ap[0][1] * out_ap.ap[1][1] == round_up_to_multiple(num_idxs, 128)
    assert in_ap.ap[0][0] == elem_step
    stride_bytes = elem_step * mybir.dt.size(in_ap.dtype)
    stride_bytes_256 = exact_div(stride_bytes, 256)
    assert stride_bytes_256 < 256

    _in_ap = g.lower_ap_dma(in_ap, for_custom_bir_dma=True)
    _idxs_ap = g.lower_ap(idxs_ap)
    _out_ap = g.lower_ap(out_ap)
    return g.add_instruction(
        mybir.InstDMAGatherAnt(
            name=g.bass.get_next_instruction_name(),
            ins=[*_in_ap, _idxs_ap, g.lower_val_access(g.to_reg(num_idxs_reg))],
            outs=[_out_ap],
            transpose=False,
            num_idxs=num_idxs,
            elem_size=elem_size,
            stride_bytes_256=stride_bytes_256,
            gen_mode=0,
            single_packet=True,
            queue_num=queue_num,
            sbuf_tokens_per_rank=0,
            sbuf_free_dim_per_rank=0,
            sbuf_free_dim_pad_per_rank=0,
            sbuf_byte_offset=0,
        )
    )


def make_schedule(edge_index, n_nodes, n_cores=NCORES):
    """Host-side marshaling: shard edges by dst owner, bucket by
    (dst_tile, src&1), pad each bucket to chunks of 128 with a shared
    chunk count across cores so all cores run an identical program."""
    src = edge_index[0].astype(np.int64)
    dst = edge_index[1].astype(np.int64)
    npc = n_nodes // n_cores
    T = _ceil_div(npc, P)

    counts = np.zeros((n_cores, T, NPAR), np.int64)
    percore = []
    for c in range(n_cores):
        sel = (dst >= c * npc) & (dst < (c + 1) * npc)
        s = src[sel]
        d = dst[sel] - c * npc
        t = d >> 7
        b = s & 1
        key = t * NPAR + b
        order = np.argsort(key, kind="stable")
        s, d, key = s[order], d[order], key[order]
        cnt = np.bincount(key, minlength=T * NPAR).reshape(T, NPAR)
        counts[c] = cnt
        percore.append((s, d, cnt))

    K = _ceil_div(counts.max(axis=0), P).astype(np.int64)  # [T, NPAR]
    total_chunks = int(K.sum())

    # chunk (t,b,k) is gathered by stream 2b + ((k + t) & 1)
    m_start = np.zeros((T, NPAR), np.int64)
    pos_s = np.zeros((T, NSTR), np.int64)
    S_s = np.zeros(NSTR, np.int64)
    m = 0
    for t in range(T):
        for b in range(NPAR):
            m_start[t, b] = m
            m += K[t, b]
            for par in range(2):
                s_ = 2 * b + par
                pos_s[t, s_] = S_s[s_]
                n_par = (int(K[t, b]) + (1 - ((par + t) & 1))) // 2
                S_s[s_] += n_par

    gather_groups = []
    for s_ in range(NSTR):
        sizes = []
        rem = int(S_s[s_])
        while rem > 0:
            g = min(GCHUNK, rem)
            sizes.append(g)
            rem -= g
        gather_groups.append(sizes)

    s_base = np.concatenate([[0], np.cumsum(S_s)]).astype(np.int64)
    core_data = []
    for c in range(n_cores):
        s, d, cnt = percore[c]
        off = np.concatenate([[0], np.cumsum(cnt.reshape(-1))]).astype(np.int64)
        idx_streams = [np.zeros(max(int(S_s[s_]), 1) * P, np.int16)
                       for s_ in range(NSTR)]
        dstloc = np.full(total_chunks * P, -1.0, np.float32)
        for t in range(T):
            for b in range(NPAR):
                n = int(cnt[t, b])
                o = off[t * NPAR + b]
                ss = ((s[o:o + n] >> 1) - ABASE).astype(np.int16)
                dd = (d[o:o + n] - t * P).astype(np.float32)
                for k in range(int(K[t, b])):
                    e0 = P * k
                    e1 = min(P * (k + 1), n)
                    if e1 <= e0:
                        break  # rest is padding: idx 0 = token ABASE, harmless
                    cs = ss[e0:e1].copy()
                    cdd = dd[e0:e1].copy()
                    if e1 - e0 == P and cs[-1] < 0:
                        # the SWDGE ucode trims trailing negative idxs, so a
                        # chunk may never end on one — swap a non-negative in
                        nz = np.nonzero(cs >= 0)[0]
                        assert len(nz), "all-negative gather chunk"
                        j = int(nz[0])
                        cs[j], cs[-1] = cs[-1], cs[j]
                        cdd[j], cdd[-1] = cdd[-1], cdd[j]
                    s_ = 2 * b + ((k + t) & 1)
                    pos = int(pos_s[t, s_]) + k // 2
                    q = pos * P
                    idx_streams[s_][q:q + (e1 - e0)] = cs
                    q0 = (int(s_base[s_]) + pos) * P
                    dstloc[q0:q0 + (e1 - e0)] = cdd
        idx_wrapped = []
        for s_ in range(NSTR):
            w = idx_streams[s_].reshape(-1, 16).T
            idx_wrapped.append(np.tile(w, (8, 1)).astype(np.int16))
        dst_t = dstloc.reshape(total_chunks, P).T.copy()
        core_data.append((idx_wrapped, dst_t))

    return {
        "n_nodes": n_nodes, "n_cores": n_cores, "npc": npc, "T": T,
        "K": K, "S_s": S_s.astype(np.int64), "total_chunks": total_chunks,
        "m_start": m_start, "pos_s": pos_s, "s_base": s_base,
        "gather_groups": gather_groups,
        "core_data": core_data,
    }


def build_bass(sched, has_bias=True):
    from concourse import bass, bacc, tile, mybir

    n_cores = sched["n_cores"]
    npc = sched["npc"]
    T = sched["T"]
    N = sched["n_nodes"]
    K = sched["K"]
    S_s = sched["S_s"]
    total_chunks = sched["total_chunks"]
    m_start = sched["m_start"]
    pos_s = sched["pos_s"]
    s_base = sched["s_base"]
    gather_groups = sched["gather_groups"]
    f32 = mybir.dt.float32
    bf16 = mybir.dt.bfloat16
    i16 = mybir.dt.int16
    i32 = mybir.dt.int32

    nc = bacc.Bacc("TRN2", target_bir_lowering=False, debug=False,
                   enable_asserts=True, num_devices=n_cores,
                   num_swdge_queues=4)

    embT = nc.dram_tensor("embT", [P, T * P], bf16, kind="ExternalInput")
    W1_d = nc.dram_tensor("W1", [EDIM, H], bf16, kind="ExternalInput")
    W2_d = nc.dram_tensor("W2", [H, H], bf16, kind="ExternalInput")
    b1_d = nc.dram_tensor("b1r", [1, H], bf16, kind="ExternalInput")
    b2_d = nc.dram_tensor("b2r", [1, H], bf16, kind="ExternalInput")
    dinv_d = nc.dram_tensor("dinv_t", [P, T], f32, kind="ExternalInput")
    sqd_d = nc.dram_tensor("sqd_row", [1, T * P], bf16, kind="ExternalInput")
    idx_d = [nc.dram_tensor(f"idx{s}", [P, max(int(S_s[s]), 1) * 8], i16,
                            kind="ExternalInput") for s in range(NSTR)]
    dst_d = nc.dram_tensor("dstloc", [P, max(total_chunks, 1)], bf16,
                           kind="ExternalInput")
    out_d = nc.dram_tensor("out", [npc, H], f32, kind="ExternalOutput")

    with tile.TileContext(nc) as tc:
        with tc.tile_pool(name="const", bufs=1) as constp, \
             tc.tile_pool(name="tables", bufs=1) as tablep, \
             tc.tile_pool(name="work", bufs=3) as workp, \
             tc.tile_pool(name="gath", bufs=16) as gathp, \
             tc.tile_pool(name="spool", bufs=4) as spool, \
             tc.tile_pool(name="psum", bufs=4, space="PSUM") as psump, \
             tc.tile_pool(name="psumB", bufs=1, space="PSUM") as psumBp, \
             tc.tile_pool(name="psumT", bufs=1, space="PSUM") as psumTp, \
             tc.tile_pool(name="dram", bufs=1, space="DRAM") as dramp:

            # ---- constants ----
            from concourse.masks import make_identity
            ident = constp.tile([P, P], bf16)
            make_identity(nc, ident[:])
            ident_f = constp.tile([P, P], f32)
            make_identity(nc, ident_f[:])
            iota_i = constp.tile([P, SBATCH * P], i32)
            nc.gpsimd.iota(iota_i[:], pattern=[[0, SBATCH], [1, P]],
                           base=0, channel_multiplier=0)
            iota_f = constp.tile([P, SBATCH * P], bf16)
            nc.vector.tensor_copy(iota_f[:], iota_i[:])

            W1_s = constp.tile([EDIM, H], bf16)
            nc.sync.dma_start(out=W1_s[:], in_=W1_d[:])
            W2_s = constp.tile([H, H], bf16)
            nc.sync.dma_start(out=W2_s[:], in_=W2_d[:])
            b1_b = constp.tile([1, H], bf16)
            nc.sync.dma_start(out=b1_b[:], in_=b1_d[:])
            b2_b = constp.tile([1, H], bf16)
            nc.sync.dma_start(out=b2_b[:], in_=b2_d[:])
            dinv_s = constp.tile([P, T], f32)
            nc.sync.dma_start(out=dinv_s[:], in_=dinv_d[:])
            sqd_b = constp.tile([1, T * P], bf16)
            nc.sync.dma_start(out=sqd_b[:], in_=sqd_d[:])
            zero_b = constp.tile([P, H], bf16)
            nc.vector.memset(zero_b[:], 0.0)

            # ---- persistent tables in SBUF ----
            y1_all = tablep.tile([P, T * H], bf16)
            y2_all = tablep.tile([P, T * H], bf16)
            maxS = max(int(x) for x in S_s)
            idx_all = tablep.tile([P, max(maxS, 1) * 8], i16)
            for s in range(NSTR):
                if int(S_s[s]) > 0:
                    nc.sync.dma_start(
                        out=idx_all[32 * s:32 * s + 32, :int(S_s[s]) * 8],
                        in_=idx_d[s][32 * s:32 * s + 32, :])
            dst_all = tablep.tile([P, max(total_chunks, 1)], bf16)
            nc.sync.dma_start(out=dst_all[:], in_=dst_d[:])

            # ---- DRAM staging for collectives ----
            y1_in = dramp.tile([npc, H], bf16)
            y2_in = dramp.tile([npc, H], bf16)
            # Shared pair-HBM outputs, viewed as [N/2, 256B] bf16 token rows
            y1_full = nc.dram_tensor("y1_full_sh", [N // 2, 2 * H], bf16,
                                     addr_space="Shared")
            y2_full = nc.dram_tensor("y2_full_sh", [N // 2, 2 * H], bf16,
                                     addr_space="Shared")

            def stage_write(dst_dram, src_tile, w0, nw):
                # src_tile column 0 corresponds to tile w0
                if (w0 + nw) * P <= npc:
                    nc.sync.dma_start(
                        out=dst_dram[w0 * P:(w0 + nw) * P, :]
                            .rearrange("(c p) h -> p c h", p=P),
                        in_=src_tile[:, :nw * H]
                            .rearrange("p (c h) -> p c h", h=H))
                else:
                    for i in range(nw):
                        t_ = w0 + i
                        rows = min(npc - t_ * P, P)
                        nc.sync.dma_start(
                            out=dst_dram[t_ * P:t_ * P + rows, :],
                            in_=src_tile[:rows, i * H:(i + 1) * H])

            # ---- phase 1: y1 = dinv * (emb @ W1) for own nodes, batched ----
            ECH = 16
            for t0 in range(0, T, ECH):
                ntile = min(ECH, T - t0)
                xt = workp.tile([P, ECH * P], bf16, tag="embT")
                nc.sync.dma_start(out=xt[:, :ntile * P],
                                  in_=embT[:, t0 * P:(t0 + ntile) * P])
                for w0 in range(t0, t0 + ntile, WB):
                    nw = min(WB, t0 + ntile - w0)
                    ps = psumBp.tile([P, WB * H], f32, tag="psb")
                    for i in range(nw):
                        nc.tensor.matmul(
                            ps[:, i * H:(i + 1) * H],
                            lhsT=xt[:, (w0 - t0 + i) * P:(w0 - t0 + i + 1) * P],
                            rhs=W1_s[:], start=True, stop=True)
                    ys = y1_all[:, w0 * H:(w0 + nw) * H]
                    nc.vector.tensor_tensor(
                        out=ys.rearrange("p (c h) -> p c h", h=H),
                        in0=ps[:, :nw * H].rearrange("p (c h) -> p c h", h=H),
                        in1=dinv_s[:, w0:w0 + nw].rearrange("p (c u) -> p c u", u=1)
                            .to_broadcast([P, nw, H]),
                        op=mybir.AluOpType.mult)
                    stage_write(y1_in, y1_all[:, w0 * H:], w0, nw)

            # ---- phase 2: AllGather layer-1 table (bf16) ----
            nc.gpsimd.collective_compute(
                "AllGather", mybir.AluOpType.bypass,
                replica_groups=[list(range(n_cores))],
                ins=[y1_in.opt()],
                outs=[y1_full[:, :].opt()],
            )

            # ---- aggregation pass (used for both layers) ----
            def aggregation(src_table, y_own, b_s, layer):
                groups = [[] for _ in range(NSTR)]   # (start, size, tile) x<=2
                next_group = [0] * NSTR
                scache = [[] for _ in range(NSTR)]   # (lo, view) x<=2
                prefix = [np.concatenate([[0], np.cumsum(gather_groups[s])])
                          .astype(int) for s in range(NSTR)]

                def ensure_gather(b, pos):
                    while (not groups[b]
                           or pos >= groups[b][-1][0] + groups[b][-1][1]):
                        g = next_group[b]
                        start = int(prefix[b][g])
                        size = gather_groups[b][g]
                        gt = gathp.tile([P, GCHUNK, H], bf16, tag=f"g{b}")
                        par = b // 2
                        my_dma_gather(
                            nc.gpsimd,
                            out_ap=gt[:, :size, :],
                            in_ap=src_table[ABASE:N // 2, par * H:(par + 1) * H],
                            idxs_ap=idx_all[:, start * 8:(start + size) * 8],
                            num_idxs=size * P,
                            num_idxs_reg=size * P,
                            elem_size=H,
                            elem_step=2 * H,
                            queue_num=b,
                        )
                        groups[b].append((start, size, gt))
                        if len(groups[b]) > 2:
                            groups[b].pop(0)
                        next_group[b] += 1
                    for st, sz, gt in groups[b]:
                        if st <= pos < st + sz:
                            return gt[:, pos - st, :]
                    raise AssertionError(f"gather pos {pos} fell out of window")

                def ensure_s(s_, pos):
                    lo = (pos // SBATCH) * SBATCH
                    for l_, view in scache[s_]:
                        if l_ == lo:
                            return view[:, (pos - lo) * P:(pos - lo + 1) * P]
                    glo = int(s_base[s_]) + lo
                    nb = min(SBATCH, int(S_s[s_]) - lo)
                    st = spool.tile([P, SBATCH * P], bf16, tag=f"S{s_}")
                    dl = dst_all[:, glo:glo + nb]
                    dl3 = dl.rearrange("p (c u) -> p c u", u=1)
                    nc.vector.tensor_tensor(
                        out=st[:, :nb * P].rearrange("p (c j) -> p c j", j=P),
                        in0=iota_f[:, :nb * P].rearrange("p (c j) -> p c j", j=P),
                        in1=dl3.to_broadcast([P, nb, P]),
                        op=mybir.AluOpType.is_equal)
                    scache[s_].append((lo, st))
                    if len(scache[s_]) > 2:
                        scache[s_].pop(0)
                    return st[:, (pos - lo) * P:(pos - lo + 1) * P]

                def tile_chunks(t):
                    nch = [0] * NSTR
                    for b in range(NPAR):
                        for par in range(2):
                            s_ = 2 * b + par
                            nch[s_] = (int(K[t, b])
                                       + (1 - ((par + t) & 1))) // 2
                    return nch

                for t0 in range(0, T, 2):
                    pair = [t for t in (t0, t0 + 1) if t < T]
                    ps = {}
                    nch = {}
                    used = {}
                    left = {}
                    first = {}
                    rr = {}
                    for t in pair:
                        ps[t] = psump.tile([P, H], f32, tag="ps",
                                           name=f"ps_{t & 3}")
                        nch[t] = tile_chunks(t)
                        used[t] = [0] * NSTR
                        left[t] = sum(nch[t])
                        first[t] = True
                        rr[t] = 0
                    # interleave the two tiles' chains so consecutive PE
                    # matmuls hit different PSUM banks and pipeline
                    while any(left[t] > 0 for t in pair):
                        for t in pair:
                            if left[t] == 0:
                                continue
                            s_ = None
                            for _ in range(NSTR):
                                cand = rr[t]
                                rr[t] = (rr[t] + 1) % NSTR
                                if used[t][cand] < nch[t][cand]:
                                    s_ = cand
                                    break
                            if s_ is None:
                                continue
                            pos = int(pos_s[t, s_]) + used[t][s_]
                            used[t][s_] += 1
                            left[t] -= 1
                            gview = ensure_gather(s_, pos)
                            sview = ensure_s(s_, pos)
                            nc.tensor.matmul(ps[t][:], lhsT=sview, rhs=gview,
                                             start=first[t], stop=False)
                            first[t] = False
                    for t in pair:
                        nc.tensor.matmul(ps[t][:], lhsT=ident[:],
                                         rhs=y_own[:, t * H:(t + 1) * H],
                                         start=first[t], stop=not has_bias)
                        if has_bias:
                            nc.tensor.matmul(
                                ps[t][:], lhsT=sqd_b[:, t * P:(t + 1) * P],
                                rhs=b_s[:], start=False, stop=True)
                    for t in pair:
                        yield t, ps[t]

            # ---- phase 3: layer-1 aggregation + fused layer-2 table ----
            for t, ps in aggregation(y1_full, y1_all, b1_b, 1):
                h1 = workp.tile([P, H], f32, tag="h1")
                nc.scalar.activation(h1[:], ps[:],
                                     mybir.ActivationFunctionType.Relu,
                                     scale=dinv_s[:, t:t + 1])
                pT = psumTp.tile([H, P], f32)
                nc.tensor.transpose(pT[:], h1[:], ident_f[:])
                h1T = workp.tile([H, P], bf16, tag="h1T")
                nc.vector.tensor_copy(h1T[:], pT[:])
                ps2 = psumBp.tile([P, H], f32, tag="ps2")
                nc.tensor.matmul(ps2[:], lhsT=h1T[:], rhs=W2_s[:],
                                 start=True, stop=True)
                y2s = y2_all[:, t * H:(t + 1) * H]
                nc.vector.tensor_scalar_mul(y2s, ps2[:], dinv_s[:, t:t + 1])
                if t % WB == WB - 1 or t == T - 1:
                    w0 = (t // WB) * WB
                    stage_write(y2_in, y2_all[:, w0 * H:], w0, t - w0 + 1)

            # ---- phase 4: AllGather layer-2 table (bf16) ----
            nc.gpsimd.collective_compute(
                "AllGather", mybir.AluOpType.bypass,
                replica_groups=[list(range(n_cores))],
                ins=[y2_in.opt()],
                outs=[y2_full[:, :].opt()],
            )

            # ---- phase 5: layer-2 aggregation -> output, batched writes ----
            otb = [None]
            for t, ps in aggregation(y2_full, y2_all, b2_b, 2):
                if t % WB == 0:
                    otb[0] = workp.tile([P, WB * H], f32, tag="ot", name="otb")
                nc.vector.tensor_scalar_mul(
                    otb[0][:, (t % WB) * H:(t % WB + 1) * H],
                    ps[:], dinv_s[:, t:t + 1])
                if t % WB == WB - 1 or t == T - 1:
                    w0 = (t // WB) * WB
                    nw = t - w0 + 1
                    if (w0 + nw) * P <= npc:
                        nc.sync.dma_start(
                            out=out_d[w0 * P:(w0 + nw) * P, :]
                                .rearrange("(c p) h -> p c h", p=P),
                            in_=otb[0][:, :nw * H]
                                .rearrange("p (c h) -> p c h", h=H))
                    else:
                        for i in range(nw):
                            t_ = w0 + i
                            rows = min(npc - t_ * P, P)
                            nc.sync.dma_start(
                                out=out_d[t_ * P:t_ * P + rows, :],
                                in_=otb[0][:rows, i * H:(i + 1) * H])

    nc.compile()
    return nc


def make_inputs(sched, emb_weight, W1, b1, W2, b2, deg):
    n_cores = sched["n_cores"]
    npc = sched["npc"]
    T = sched["T"]
    dinv = (1.0 / np.sqrt(deg.astype(np.float64))).astype(np.float32)
    sqd = np.sqrt(deg.astype(np.float64)).astype(np.float32)
    in_maps = []
    for c in range(n_cores):
        lo, hi = c * npc, (c + 1) * npc
        embT = np.zeros((P, T * P), np.float32)
        embT[:, :npc] = emb_weight[lo:hi].T
        tmp = np.zeros(T * P, np.float32)
        tmp[:npc] = dinv[lo:hi]
        dinv_t = np.ascontiguousarray(tmp.reshape(T, P).T)
        sqd_row = np.zeros((1, T * P), np.float32)
        sqd_row[0, :npc] = sqd[lo:hi]
        idx_wrapped, dst_t = sched["core_data"][c]
        m = {
            "embT": embT.astype(ml_dtypes.bfloat16),
            "W1": W1.astype(ml_dtypes.bfloat16),
            "W2": W2.astype(ml_dtypes.bfloat16),
            "b1r": b1.reshape(1, -1).astype(ml_dtypes.bfloat16),
            "b2r": b2.reshape(1, -1).astype(ml_dtypes.bfloat16),
            "dinv_t": dinv_t,
            "sqd_row": sqd_row.astype(ml_dtypes.bfloat16),
            "dstloc": dst_t.astype(ml_dtypes.bfloat16),
        }
        for s in range(NSTR):
            iw = idx_wrapped[s]
            if iw.shape[1] == 0:
                iw = np.zeros((P, 8), np.int16)
            m[f"idx{s}"] = iw
        in_maps.append(m)
    return in_maps


def run(edge_index, emb_weight, W1, b1, W2, b2, n_nodes=None, trace=False):
    from concourse import bass_utils
    n_nodes = n_nodes if n_nodes is not None else emb_weight.shape[0]
    sched = make_schedule(np.asarray(edge_index), n_nodes)
    has_bias = bool(np.any(np.asarray(b1)) or np.any(np.asarray(b2)))
    key = ("gnn4", n_nodes, int(sched["total_chunks"]), has_bias,
           tuple(int(x) for x in sched["S_s"]))
    if key not in _COMPILED:
        _COMPILED[key] = build_bass(sched, has_bias)
    nc = _COMPILED[key]
    deg = np.bincount(np.asarray(edge_index)[1], minlength=n_nodes).astype(np.float32) + 1.0
    in_maps = make_inputs(sched, np.asarray(emb_weight), np.asarray(W1),
                          np.asarray(b1), np.asarray(W2), np.asarray(b2), deg)
    res = bass_utils.run_bass_kernel_spmd(
        nc, in_maps, core_ids=list(range(sched["n_cores"])), trace=trace)
    npc = sched["npc"]
    out = np.concatenate([res.results[c]["out"] for c in range(sched["n_cores"])],
                         axis=0)
    return out[:n_nodes], res


def kernel(edge_index, emb_weight, W1, b1, W2, b2):
    out, _ = run(edge_index, emb_weight, W1, b1, W2, b2)
    return out


# revision 17
# speedup vs baseline: 1.0770x; 1.0770x over previous
"""Trainium2 Bass kernel for a 2-layer GCN (NextHopGNN).

Distribution: destination-node sharding across 8 NeuronCores. Each core owns
N/8 destination nodes and all edges pointing into them. Per layer:
  1. each core computes its slice of the scaled feature table
     y = dinv * (x @ W) in bf16 (PE matmuls, batched 8-tile PSUM groups)
  2. AllGather (bf16) -> every core holds the full [N, 64] table in HBM
  3. per 128-node dst tile: dma_gather the edge source rows as 128-byte
     bf16 descriptors (table viewed as [N/2, 256B] token pairs; src&1
     selects a 64-column half; int16 idx = (src>>1) - 32768), build
     one-hot scatter matrices on the vector engine (batched iota ==
     dstloc), and accumulate S^T @ G in PSUM on the tensor engine.
     Self-loop added via an identity matmul from the resident bf16 table.
Layer-1 epilogue also computes the layer-2 table tile (relu -> transpose ->
@W2 -> scale) so layer 2 only needs the second AllGather + aggregation.

The SWDGE gather drain is descriptor-rate-bound (~8ns/descriptor/queue,
4 queues), so the schedule minimizes padded descriptor count (2 parity
buckets/tile) and keeps deep gather lookahead so the PE never starves.
"""
import sys
import os
import numpy as np
import ml_dtypes

sys.path.insert(0, "/opt/trn_rl_repo")

P = 128
H = 64
EDIM = 128
NCORES = 8
NPAR = 2            # src&1 parity buckets
NSTR = 4            # gather streams (= SWDGE queues), 2 per parity
ABASE = 32768       # token bias: idx = (src>>1) - ABASE, tokens N/2 = 50000
GCHUNK = 8          # max chunks (of 128 idxs) per dma_gather instruction
                    # (>1024 idxs per instruction crashes the SWDGE ucode)
SBATCH = 8          # chunks per batched one-hot build
WB = 8              # tiles per batched table-build / write group

_COMPILED = {}


def _ceil_div(a, b):
    return (a + b - 1) // b


def my_dma_gather(g, out_ap, in_ap, idxs_ap, num_idxs, num_idxs_reg,
                  elem_size, elem_step, queue_num=0):
    """Replica of BassGpSimd.dma_gather (non-transpose, DRAM source) minus
    the elem_size_bytes%256 assert. The real encoding constraint is the
    256B-aligned source row stride (stride_bytes_256); 128B (bf16 x64) and
    64B (fp8 x64) elem sizes were verified bit-exact on HW."""
    from concourse import mybir, ap_utils
    from concourse.bass import exact_div, round_up_to_multiple
    assert idxs_ap.dtype == mybir.dt.int16
    assert in_ap.dtype == out_ap.dtype
    assert ap_utils.ap_is_contiguous(out_ap.ap[1:])
    assert ap_utils.ap_is_contiguous(idxs_ap.ap[1:])
    assert in_ap.ap[-1][1] == out_ap.ap[-1][1] == elem_size
    assert out_ap.ap[0][1] * out_ap.ap[1][1] == round_up_to_multiple(num_idxs, 128)
    assert in_ap.ap[0][0] == elem_step
    stride_bytes = elem_step * mybir.dt.size(in_ap.dtype)
    stride_bytes_256 = exact_div(stride_bytes, 256)
    assert stride_bytes_256 < 256

    _in_ap = g.lower_ap_dma(in_ap, for_custom_bir_dma=True)
    _idxs_ap = g.lower_ap(idxs_ap)
    _out_ap = g.lower_ap(out_ap)
    return g.add_instruction(
        mybir.InstDMAGatherAnt(
            name=g.bass.get_next_instruction_name(),
            ins=[*_in_ap, _idxs_ap, g.lower_val_access(g.to_reg(num_idxs_reg))],
            outs=[_out_ap],
            transpose=False,
            num_idxs=num_idxs,
            elem_size=elem_size,
            stride_bytes_256=stride_bytes_256,
            gen_mode=0,
            single_packet=True,
            queue_num=queue_num,
            sbuf_tokens_per_rank=0,
            sbuf_free_dim_per_rank=0,
            sbuf_free_dim_pad_per_rank=0,
            sbuf_byte_offset=0,
        )
    )


def make_schedule(edge_index, n_nodes, n_cores=NCORES):
    """Host-side marshaling: shard edges by dst owner, bucket by
    (dst_tile, src&1), pad each bucket to chunks of 128 with a shared
    chunk count across cores so all cores run an identical program."""
    src = edge_index[0].astype(np.int64)
    dst = edge_index[1].astype(np.int64)
    npc = n_nodes // n_cores
    T = _ceil_div(npc, P)

    counts = np.zeros((n_cores, T, NPAR), np.int64)
    percore = []
    for c in range(n_cores):
        sel = (dst >= c * npc) & (dst < (c + 1) * npc)
        s = src[sel]
        d = dst[sel] - c * npc
        t = d >> 7
        b = s & 1
        key = t * NPAR + b
        order = np.argsort(key, kind="stable")
        s, d, key = s[order], d[order], key[order]
        cnt = np.bincount(key, minlength=T * NPAR).reshape(T, NPAR)
        counts[c] = cnt
        percore.append((s, d, cnt))

    K = _ceil_div(counts.max(axis=0), P).astype(np.int64)  # [T, NPAR]
    total_chunks = int(K.sum())

    # chunk (t,b,k) is gathered by stream 2b + ((k + t) & 1)
    m_start = np.zeros((T, NPAR), np.int64)
    pos_s = np.zeros((T, NSTR), np.int64)
    S_s = np.zeros(NSTR, np.int64)
    m = 0
    for t in range(T):
        for b in range(NPAR):
            m_start[t, b] = m
            m += K[t, b]
            for par in range(2):
                s_ = 2 * b + par
                pos_s[t, s_] = S_s[s_]
                n_par = (int(K[t, b]) + (1 - ((par + t) & 1))) // 2
                S_s[s_] += n_par

    gather_groups = []
    for s_ in range(NSTR):
        sizes = []
        rem = int(S_s[s_])
        while rem > 0:
            g = min(GCHUNK, rem)
            sizes.append(g)
            rem -= g
        gather_groups.append(sizes)

    s_base = np.concatenate([[0], np.cumsum(S_s)]).astype(np.int64)
    core_data = []
    for c in range(n_cores):
        s, d, cnt = percore[c]
        off = np.concatenate([[0], np.cumsum(cnt.reshape(-1))]).astype(np.int64)
        idx_streams = [np.zeros(max(int(S_s[s_]), 1) * P, np.int16)
                       for s_ in range(NSTR)]
        dstloc = np.full(total_chunks * P, -1.0, np.float32)
        for t in range(T):
            for b in range(NPAR):
                n = int(cnt[t, b])
                o = off[t * NPAR + b]
                ss = ((s[o:o + n] >> 1) - ABASE).astype(np.int16)
                dd = (d[o:o + n] - t * P).astype(np.float32)
                for k in range(int(K[t, b])):
                    e0 = P * k
                    e1 = min(P * (k + 1), n)
                    if e1 <= e0:
                        break  # rest is padding: idx 0 = token ABASE, harmless
                    cs = ss[e0:e1].copy()
                    cdd = dd[e0:e1].copy()
                    if e1 - e0 == P and cs[-1] < 0:
                        # the SWDGE ucode trims trailing negative idxs, so a
                        # chunk may never end on one — swap a non-negative in
                        nz = np.nonzero(cs >= 0)[0]
                        assert len(nz), "all-negative gather chunk"
                        j = int(nz[0])
                        cs[j], cs[-1] = cs[-1], cs[j]
                        cdd[j], cdd[-1] = cdd[-1], cdd[j]
                    s_ = 2 * b + ((k + t) & 1)
                    pos = int(pos_s[t, s_]) + k // 2
                    q = pos * P
                    idx_streams[s_][q:q + (e1 - e0)] = cs
                    q0 = (int(s_base[s_]) + pos) * P
                    dstloc[q0:q0 + (e1 - e0)] = cdd
        idx_wrapped = []
        for s_ in range(NSTR):
            w = idx_streams[s_].reshape(-1, 16).T
            idx_wrapped.append(np.tile(w, (8, 1)).astype(np.int16))
        dst_t = dstloc.reshape(total_chunks, P).T.copy()
        core_data.append((idx_wrapped, dst_t))

    return {
        "n_nodes": n_nodes, "n_cores": n_cores, "npc": npc, "T": T,
        "K": K, "S_s": S_s.astype(np.int64), "total_chunks": total_chunks,
        "m_start": m_start, "pos_s": pos_s, "s_base": s_base,
        "gather_groups": gather_groups,
        "core_data": core_data,
    }


def build_bass(sched, has_bias=True):
    from concourse import bass, bacc, tile, mybir

    n_cores = sched["n_cores"]
    npc = sched["npc"]
    T = sched["T"]
    N = sched["n_nodes"]
    K = sched["K"]
    S_s = sched["S_s"]
    total_chunks = sched["total_chunks"]
    m_start = sched["m_start"]
    pos_s = sched["pos_s"]
    s_base = sched["s_base"]
    gather_groups = sched["gather_groups"]
    f32 = mybir.dt.float32
    bf16 = mybir.dt.bfloat16
    i16 = mybir.dt.int16
    i32 = mybir.dt.int32

    nc = bacc.Bacc("TRN2", target_bir_lowering=False, debug=False,
                   enable_asserts=True, num_devices=n_cores,
                   num_swdge_queues=4)

    embT = nc.dram_tensor("embT", [P, T * P], bf16, kind="ExternalInput")
    W1_d = nc.dram_tensor("W1", [EDIM, H], bf16, kind="ExternalInput")
    W2_d = nc.dram_tensor("W2", [H, H], bf16, kind="ExternalInput")
    b1_d = nc.dram_tensor("b1r", [1, H], bf16, kind="ExternalInput")
    b2_d = nc.dram_tensor("b2r", [1, H], bf16, kind="ExternalInput")
    dinv_d = nc.dram_tensor("dinv_t", [P, T], f32, kind="ExternalInput")
    sqd_d = nc.dram_tensor("sqd_row", [1, T * P], bf16, kind="ExternalInput")
    idx_d = [nc.dram_tensor(f"idx{s}", [P, max(int(S_s[s]), 1) * 8], i16,
                            kind="ExternalInput") for s in range(NSTR)]
    dst_d = nc.dram_tensor("dstloc", [P, max(total_chunks, 1)], bf16,
                           kind="ExternalInput")
    out_d = nc.dram_tensor("out", [npc, H], f32, kind="ExternalOutput")

    with tile.TileContext(nc) as tc:
        with tc.tile_pool(name="const", bufs=1) as constp, \
             tc.tile_pool(name="tables", bufs=1) as tablep, \
             tc.tile_pool(name="work", bufs=3) as workp, \
             tc.tile_pool(name="gath", bufs=16) as gathp, \
             tc.tile_pool(name="spool", bufs=4) as spool, \
             tc.tile_pool(name="psum", bufs=4, space="PSUM") as psump, \
             tc.tile_pool(name="psumB", bufs=1, space="PSUM") as psumBp, \
             tc.tile_pool(name="psumT", bufs=1, space="PSUM") as psumTp, \
             tc.tile_pool(name="dram", bufs=1, space="DRAM") as dramp:

            # ---- constants ----
            from concourse.masks import make_identity
            ident = constp.tile([P, P], bf16)
            make_identity(nc, ident[:])
            ident_f = constp.tile([P, P], f32)
            make_identity(nc, ident_f[:])
            iota_i = constp.tile([P, SBATCH * P], i32)
            nc.gpsimd.iota(iota_i[:], pattern=[[0, SBATCH], [1, P]],
                           base=0, channel_multiplier=0)
            iota_f = constp.tile([P, SBATCH * P], bf16)
            nc.vector.tensor_copy(iota_f[:], iota_i[:])

            W1_s = constp.tile([EDIM, H], bf16)
            nc.sync.dma_start(out=W1_s[:], in_=W1_d[:])
            W2_s = constp.tile([H, H], bf16)
            nc.sync.dma_start(out=W2_s[:], in_=W2_d[:])
            b1_b = constp.tile([1, H], bf16)
            nc.sync.dma_start(out=b1_b[:], in_=b1_d[:])
            b2_b = constp.tile([1, H], bf16)
            nc.sync.dma_start(out=b2_b[:], in_=b2_d[:])
            dinv_s = constp.tile([P, T], f32)
            nc.sync.dma_start(out=dinv_s[:], in_=dinv_d[:])
            sqd_b = constp.tile([1, T * P], bf16)
            nc.sync.dma_start(out=sqd_b[:], in_=sqd_d[:])
            zero_b = constp.tile([P, H], bf16)
            nc.vector.memset(zero_b[:], 0.0)

            # ---- persistent tables in SBUF ----
            y1_all = tablep.tile([P, T * H], bf16)
            y2_all = tablep.tile([P, T * H], bf16)
            maxS = max(int(x) for x in S_s)
            idx_all = tablep.tile([P, max(maxS, 1) * 8], i16)
            for s in range(NSTR):
                if int(S_s[s]) > 0:
                    nc.sync.dma_start(
                        out=idx_all[32 * s:32 * s + 32, :int(S_s[s]) * 8],
                        in_=idx_d[s][32 * s:32 * s + 32, :])
            dst_all = tablep.tile([P, max(total_chunks, 1)], bf16)
            nc.sync.dma_start(out=dst_all[:], in_=dst_d[:])

            # ---- DRAM staging for collectives ----
            y1_in = dramp.tile([npc, H], bf16)
            y2_in = dramp.tile([npc, H], bf16)
            # Shared pair-HBM outputs, viewed as [N/2, 256B] bf16 token rows
            y1_full = nc.dram_tensor("y1_full_sh", [N // 2, 2 * H], bf16,
                                     addr_space="Shared")
            y2_full = nc.dram_tensor("y2_full_sh", [N // 2, 2 * H], bf16,
                                     addr_space="Shared")

            def stage_write(dst_dram, src_tile, w0, nw):
                # src_tile column 0 corresponds to tile w0
                if (w0 + nw) * P <= npc:
                    nc.sync.dma_start(
                        out=dst_dram[w0 * P:(w0 + nw) * P, :]
                            .rearrange("(c p) h -> p c h", p=P),
                        in_=src_tile[:, :nw * H]
                            .rearrange("p (c h) -> p c h", h=H))
                else:
                    for i in range(nw):
                        t_ = w0 + i
                        rows = min(npc - t_ * P, P)
                        nc.sync.dma_start(
                            out=dst_dram[t_ * P:t_ * P + rows, :],
                            in_=src_tile[:rows, i * H:(i + 1) * H])

            # ---- phase 1: y1 = dinv * (emb @ W1) for own nodes, batched ----
            ECH = 16
            for t0 in range(0, T, ECH):
                ntile = min(ECH, T - t0)
                xt = workp.tile([P, ECH * P], bf16, tag="embT")
                nc.sync.dma_start(out=xt[:, :ntile * P],
                                  in_=embT[:, t0 * P:(t0 + ntile) * P])
                for w0 in range(t0, t0 + ntile, WB):
                    nw = min(WB, t0 + ntile - w0)
                    ps = psumBp.tile([P, WB * H], f32, tag="psb")
                    for i in range(nw):
                        nc.tensor.matmul(
                            ps[:, i * H:(i + 1) * H],
                            lhsT=xt[:, (w0 - t0 + i) * P:(w0 - t0 + i + 1) * P],
                            rhs=W1_s[:], start=True, stop=True)
                    ys = y1_all[:, w0 * H:(w0 + nw) * H]
                    nc.vector.tensor_tensor(
                        out=ys.rearrange("p (c h) -> p c h", h=H),
                        in0=ps[:, :nw * H].rearrange("p (c h) -> p c h", h=H),
                        in1=dinv_s[:, w0:w0 + nw].rearrange("p (c u) -> p c u", u=1)
                            .to_broadcast([P, nw, H]),
                        op=mybir.AluOpType.mult)
                    stage_write(y1_in, y1_all[:, w0 * H:], w0, nw)

            # ---- phase 2: AllGather layer-1 table (bf16) ----
            nc.gpsimd.collective_compute(
                "AllGather", mybir.AluOpType.bypass,
                replica_groups=[list(range(n_cores))],
                ins=[y1_in.opt()],
                outs=[y1_full[:, :].opt()],
            )

            # ---- aggregation pass (used for both layers) ----
            def aggregation(src_table, y_own, b_s, layer):
                groups = [[] for _ in range(NSTR)]   # (start, size, tile) x<=2
                next_group = [0] * NSTR
                scache = [[] for _ in range(NSTR)]   # (lo, view) x<=2
                prefix = [np.concatenate([[0], np.cumsum(gather_groups[s])])
                          .astype(int) for s in range(NSTR)]

                def ensure_gather(b, pos):
                    while (not groups[b]
                           or pos >= groups[b][-1][0] + groups[b][-1][1]):
                        g = next_group[b]
                        start = int(prefix[b][g])
                        size = gather_groups[b][g]
                        gt = gathp.tile([P, GCHUNK, H], bf16, tag=f"g{b}")
                        par = b // 2
                        my_dma_gather(
                            nc.gpsimd,
                            out_ap=gt[:, :size, :],
                            in_ap=src_table[ABASE:N // 2, par * H:(par + 1) * H],
                            idxs_ap=idx_all[:, start * 8:(start + size) * 8],
                            num_idxs=size * P,
                            num_idxs_reg=size * P,
                            elem_size=H,
                            elem_step=2 * H,
                            queue_num=b,
                        )
                        groups[b].append((start, size, gt))
                        if len(groups[b]) > 2:
                            groups[b].pop(0)
                        next_group[b] += 1
                    for st, sz, gt in groups[b]:
                        if st <= pos < st + sz:
                            return gt[:, pos - st, :]
                    raise AssertionError(f"gather pos {pos} fell out of window")

                def ensure_s(s_, pos):
                    lo = (pos // SBATCH) * SBATCH
                    for l_, view in scache[s_]:
                        if l_ == lo:
                            return view[:, (pos - lo) * P:(pos - lo + 1) * P]
                    glo = int(s_base[s_]) + lo
                    nb = min(SBATCH, int(S_s[s_]) - lo)
                    st = spool.tile([P, SBATCH * P], bf16, tag=f"S{s_}")
                    dl = dst_all[:, glo:glo + nb]
                    dl3 = dl.rearrange("p (c u) -> p c u", u=1)
                    nc.vector.tensor_tensor(
                        out=st[:, :nb * P].rearrange("p (c j) -> p c j", j=P),
                        in0=iota_f[:, :nb * P].rearrange("p (c j) -> p c j", j=P),
                        in1=dl3.to_broadcast([P, nb, P]),
                        op=mybir.AluOpType.is_equal)
                    scache[s_].append((lo, st))
                    if len(scache[s_]) > 2:
                        scache[s_].pop(0)
                    return st[:, (pos - lo) * P:(pos - lo + 1) * P]

                def tile_chunks(t):
                    nch = [0] * NSTR
                    for b in range(NPAR):
                        for par in range(2):
                            s_ = 2 * b + par
                            nch[s_] = (int(K[t, b])
                                       + (1 - ((par + t) & 1))) // 2
                    return nch

                for t0 in range(0, T, 2):
                    pair = [t for t in (t0, t0 + 1) if t < T]
                    ps = {}
                    nch = {}
                    used = {}
                    left = {}
                    first = {}
                    rr = {}
                    for t in pair:
                        ps[t] = psump.tile([P, H], f32, tag="ps",
                                           name=f"ps_{t & 3}")
                        nch[t] = tile_chunks(t)
                        used[t] = [0] * NSTR
                        left[t] = sum(nch[t])
                        first[t] = True
                        rr[t] = 0
                    # interleave the two tiles' chains so consecutive PE
                    # matmuls hit different PSUM banks and pipeline
                    while any(left[t] > 0 for t in pair):
                        for t in pair:
                            if left[t] == 0:
                                continue
                            s_ = None
                            for _ in range(NSTR):
                                cand = rr[t]
                                rr[t] = (rr[t] + 1) % NSTR
                                if used[t][cand] < nch[t][cand]:
                                    s_ = cand
                                    break
                            if s_ is None:
                                continue
                            pos = int(pos_s[t, s_]) + used[t][s_]
                            used[t][s_] += 1
                            left[t] -= 1
                            gview = ensure_gather(s_, pos)
                            sview = ensure_s(s_, pos)
                            nc.tensor.matmul(ps[t][:], lhsT=sview, rhs=gview,
                                             start=first[t], stop=False)
                            first[t] = False
                    for t in pair:
                        nc.tensor.matmul(ps[t][:], lhsT=ident[:],
                                         rhs=y_own[:, t * H:(t + 1) * H],
                                         start=first[t], stop=not has_bias)
                        if has_bias:
                            nc.tensor.matmul(
                                ps[t][:], lhsT=sqd_b[:, t * P:(t + 1) * P],
                                rhs=b_s[:], start=False, stop=True)
                    for t in pair:
                        yield t, ps[t]

            # ---- phase 3: layer-1 aggregation + fused layer-2 table ----
            for t, ps in aggregation(y1_full, y1_all, b1_b, 1):
                h1 = workp.tile([P, H], f32, tag="h1")
                nc.scalar.activation(h1[:], ps[:],
                                     mybir.ActivationFunctionType.Relu,
                                     scale=dinv_s[:, t:t + 1])
                pT = psumTp.tile([H, P], f32)
                nc.tensor.transpose(pT[:], h1[:], ident_f[:])
                h1T = workp.tile([H, P], bf16, tag="h1T")
                nc.vector.tensor_copy(h1T[:], pT[:])
                ps2 = psumBp.tile([P, H], f32, tag="ps2")
                nc.tensor.matmul(ps2[:], lhsT=h1T[:], rhs=W2_s[:],
                                 start=True, stop=True)
                y2s = y2_all[:, t * H:(t + 1) * H]
                nc.vector.tensor_scalar_mul(y2s, ps2[:], dinv_s[:, t:t + 1])
                if t % WB == WB - 1 or t == T - 1:
                    w0 = (t // WB) * WB
                    stage_write(y2_in, y2_all[:, w0 * H:], w0, t - w0 + 1)

            # ---- phase 4: AllGather layer-2 table (bf16) ----
            nc.gpsimd.collective_compute(
                "AllGather", mybir.AluOpType.bypass,
                replica_groups=[list(range(n_cores))],
                ins=[y2_in.opt()],
                outs=[y2_full[:, :].opt()],
            )

            # ---- phase 5: layer-2 aggregation -> output, batched writes ----
            otb = [None]
            for t, ps in aggregation(y2_full, y2_all, b2_b, 2):
                if t % WB == 0:
                    otb[0] = workp.tile([P, WB * H], f32, tag="ot", name="otb")
                nc.vector.tensor_scalar_mul(
                    otb[0][:, (t % WB) * H:(t % WB + 1) * H],
                    ps[:], dinv_s[:, t:t + 1])
                if t % WB == WB - 1 or t == T - 1:
                    w0 = (t // WB) * WB
                    nw = t - w0 + 1
                    if (w0 + nw) * P <= npc:
                        nc.sync.dma_start(
                            out=out_d[w0 * P:(w0 + nw) * P, :]
                                .rearrange("(c p) h -> p c h", p=P),
                            in_=otb[0][:, :nw * H]
                                .rearrange("p (c h) -> p c h", h=H))
                    else:
                        for i in range(nw):
                            t_ = w0 + i
                            rows = min(npc - t_ * P, P)
                            nc.sync.dma_start(
                                out=out_d[t_ * P:t_ * P + rows, :],
                                in_=otb[0][:rows, i * H:(i + 1) * H])

    nc.compile()
    return nc


def make_inputs(sched, emb_weight, W1, b1, W2, b2, deg):
    n_cores = sched["n_cores"]
    npc = sched["npc"]
    T = sched["T"]
    dinv = (1.0 / np.sqrt(deg.astype(np.float64))).astype(np.float32)
    sqd = np.sqrt(deg.astype(np.float64)).astype(np.float32)
    in_maps = []
    for c in range(n_cores):
        lo, hi = c * npc, (c + 1) * npc
        embT = np.zeros((P, T * P), np.float32)
        embT[:, :npc] = emb_weight[lo:hi].T
        tmp = np.zeros(T * P, np.float32)
        tmp[:npc] = dinv[lo:hi]
        dinv_t = np.ascontiguousarray(tmp.reshape(T, P).T)
        sqd_row = np.zeros((1, T * P), np.float32)
        sqd_row[0, :npc] = sqd[lo:hi]
        idx_wrapped, dst_t = sched["core_data"][c]
        m = {
            "embT": embT.astype(ml_dtypes.bfloat16),
            "W1": W1.astype(ml_dtypes.bfloat16),
            "W2": W2.astype(ml_dtypes.bfloat16),
            "b1r": b1.reshape(1, -1).astype(ml_dtypes.bfloat16),
            "b2r": b2.reshape(1, -1).astype(ml_dtypes.bfloat16),
            "dinv_t": dinv_t,
            "sqd_row": sqd_row.astype(ml_dtypes.bfloat16),
            "dstloc": dst_t.astype(ml_dtypes.bfloat16),
        }
        for s in range(NSTR):
            iw = idx_wrapped[s]
            if iw.shape[1] == 0:
                iw = np.zeros((P, 8), np.int16)
            m[f"idx{s}"] = iw
        in_maps.append(m)
    return in_maps


def run(edge_index, emb_weight, W1, b1, W2, b2, n_nodes=None, trace=False):
    from concourse import bass_utils
    n_nodes = n_nodes if n_nodes is not None else emb_weight.shape[0]
    sched = make_schedule(np.asarray(edge_index), n_nodes)
    has_bias = bool(np.any(np.asarray(b1)) or np.any(np.asarray(b2)))
    key = ("gnn4", n_nodes, int(sched["total_chunks"]), has_bias,
           tuple(int(x) for x in sched["S_s"]))
    if key not in _COMPILED:
        _COMPILED[key] = build_bass(sched, has_bias)
    nc = _COMPILED[key]
    deg = np.bincount(np.asarray(edge_index)[1], minlength=n_nodes).astype(np.float32) + 1.0
    in_maps = make_inputs(sched, np.asarray(emb_weight), np.asarray(W1),
                          np.asarray(b1), np.asarray(W2), np.asarray(b2), deg)
    res = bass_utils.run_bass_kernel_spmd(
        nc, in_maps, core_ids=list(range(sched["n_cores"])), trace=trace)
    npc = sched["npc"]
    out = np.concatenate([res.results[c]["out"] for c in range(sched["n_cores"])],
                         axis=0)
    return out[:n_nodes], res


def kernel(edge_index, emb_weight, W1, b1, W2, b2):
    out, _ = run(edge_index, emb_weight, W1, b1, W2, b2)
    return out
